# revision 37
# baseline (speedup 1.0000x reference)
"""BuildCost kernel for Trainium2 (Bass/Tile), 8-core SPMD.

cost[b,d,p,q,c,uv] = xpad[b,c,u,v, p+off(d,u), q+off(d,v)] * mask[b,uv,p,q]
with off(d_idx,t) = d_idx*(4-t) + 4*t  (d_idx = d - mindisp), padded border 16.

Sharding: core = b*4 + qb  (b in {0,1}, qb in {0..3} -> q columns [qb*24, qb*24+24)).

Per core inputs (host pre-laid-out in SBUF geometry):
  x3  [128 pp, (t,c,uv,qq=56)] bf16: exact 3-term split x = h + l + ll
  msk [128 p,  (u,q,v)] f32 mask slice (rows 96.. zero)
  idn [128 k,  (off=33, m=96)] bf16 shifted identities (zero cols where the
      selected row is out of range -> borders need no zeroing on device)

Per (d,u,ch): three accumulating bf16 matmuls PSUM[p, q*18+c'*9+v] +=
IDN[off(d,u)].T @ x3[t, ...]; the row shift comes from the one-hot identity
(partition-legal), the column shift off(d,v) is folded into the rhs AP via
the linear-in-v stride 60-d_idx; rhs streams q contiguously (PE-fast) while
a permuted psum out-AP lands results in (q,c',v) order (DVE-fast).
Then OUT_d[p,(q,c,u,v)] = PSUM * mask on DVE (stride-0 broadcast over c',
all APs v-contiguous), and one contiguous 2.99MB DMA per d.

The bf16 triple split is exact: h,l,ll capture disjoint 8-bit significand
ranges of the fp32 value, and the fp32 PSUM accumulation of the three
one-hot matmul results reconstructs x bit-exactly.
"""

import numpy as np

from concourse import bacc, bass, mybir
from concourse.ap import AP
from concourse.tile import TileContext
from concourse.bass_utils import run_bass_kernel_spmd

A = 9            # angular resolution
D = 9            # disparities
H = W = 96
C = 4
PAD = 16
PPAD = 128       # padded row count
QB = W // 4      # q-block per core = 24
QQ = QB + 32     # stored qq window = 56
CUV = C * A * A  # 324
XF = C * A * A * QQ   # 18144  one split-term's free size
MF = A * A * QB       # 1944   mask free size, layout (uv, q)
NOFF = 33
IDW = 128             # identity columns = 128 to trigger FWL
NT = 2                # split terms: x = h + 2^-11 * l2 (fp16, ~2-ulp exact)
IDB = IDW + NOFF - 1  # 160: all 33 shifted identities are windows of one strip
IDF = NT * IDB        # 320 free elems: set 0 value 1.0, set 1 value 2^-11
OF = QB * CUV         # 7776   per-d output tile free size
NMM = 432             # matmul moving size = 2*9*24
F32 = mybir.dt.float32
F16 = mybir.dt.float16


def off(d_idx: int, t: int) -> int:
    return d_idx * (4 - t) + 4 * t


def _mkap(base: AP, extra_off: int, dims) -> AP:
    return AP(base.tensor, base.offset + extra_off, dims)


def build_body(nc: bass.Bass, tc, out: AP, x3: AP, msk: AP, idn: AP):
    with (
        tc.tile_pool(name="cst", bufs=1) as cst,
        tc.tile_pool(name="ob", bufs=3) as ob,
        tc.tile_pool(name="ps", bufs=1, space="PSUM") as ps,
    ):
        X3 = cst.tile([PPAD, NT * XF], F16, tag="X3")
        M0 = cst.tile([PPAD, MF], F32, tag="M0")
        IDN = cst.tile([PPAD, IDF], F16, tag="IDN")
        # split the big x load into (u, t) strips so the first matmul group
        # only waits for ~1MB instead of the whole 9.3MB; u=0 strips first,
        # then the small identity/mask tiles, then the rest
        strip = [[NT * XF, PPAD], [A * A * QQ, C], [1, A * QQ]]
        nc.sync.dma_start(IDN[:], idn)
        for t in range(NT):
            o = t * XF
            nc.sync.dma_start(_mkap(X3[:], o, strip), _mkap(x3, o, strip))
        for t in range(NT):
            o = t * XF + A * QQ
            nc.sync.dma_start(_mkap(X3[:], o, strip), _mkap(x3, o, strip))
        nc.sync.dma_start(M0[:], msk)
        for u in range(2, A):
            for t in range(NT):
                o = t * XF + u * A * QQ
                nc.sync.dma_start(_mkap(X3[:], o, strip), _mkap(x3, o, strip))

        gi = 0
        for d0 in range(0, D, 2):
            dpair = [d for d in (d0, d0 + 1) if d < D]
            Ods = {
                d: ob.tile([H, OF], F32, tag="Od", name=f"Od{d}") for d in dpair
            }
            # u outer, d inner: each arriving u-strip feeds both d's worth of
            # matmuls, halving PE starvation while the input is still streaming
            for u in range(A):
                for d_idx in dpair:
                    o_u = off(d_idx, u)
                    Od = Ods[d_idx]
                    for ch in range(2):
                        # distinct cycling tags pin the 8 PSUM banks round-robin
                        P = ps.tile([PPAD, NMM], F32, tag=f"ps{gi % 8}")
                        gi += 1
                        for t in range(NT):
                            # identity set t: values 1.0 (t=0) / 2^-11 (t=1);
                            # shifted identity = sliding window of the strip
                            lhsT = _mkap(
                                IDN[:], t * IDB + o_u, [[IDF, PPAD], [1, IDW]]
                            )
                            # rhs iteration (c'(2), v(9), q(24)): q contiguous
                            # innermost (PE-fast); v-stride 60-d_idx folds off(d,v)
                            rhs = _mkap(
                                X3[:],
                                t * XF + ch * 2 * (A * A * QQ) + u * A * QQ + 4 * d_idx,
                                [[NT * XF, PPAD], [A * A * QQ, 2], [60 - d_idx, A], [1, QB]],
                            )
                            nc.tensor.matmul(
                                P[:], lhsT, rhs, start=(t == 0), stop=(t == NT - 1)
                            )
                        # all APs iterate (c', v, q), q contiguous innermost; the
                        # device OUT layout is (u, ch, c', v, q) — the host
                        # permutes to the reference (q, c, u, v) order.
                        oap = _mkap(
                            Od[:], (u * 2 + ch) * NMM,
                            [[OF, H], [A * QB, 2], [QB, A], [1, QB]],
                        )
                        pap = _mkap(P[:], 0, [[NMM, H], [A * QB, 2], [QB, A], [1, QB]])
                        map_ = _mkap(
                            M0[:], u * A * QB,
                            [[MF, H], [0, 2], [QB, A], [1, QB]],
                        )
                        nc.any.tensor_mul(oap, pap, map_)
                if u % 3 == 2:
                    # drain this u-third of each Od as soon as its 6 TTs are done
                    lo, hi = (u - 2) * 2 * NMM, (u + 1) * 2 * NMM
                    for d_idx in dpair:
                        nc.sync.dma_start(
                            _mkap(out[d_idx], lo, [[OF, H], [1, hi - lo]]),
                            _mkap(Ods[d_idx][:], lo, [[OF, H], [1, hi - lo]]),
                        )


def build_nc() -> bass.Bass:
    nc = bacc.Bacc("TRN2", target_bir_lowering=False, debug=False)
    x3 = nc.dram_tensor("x3", [PPAD, NT * XF], F16, kind="ExternalInput")
    msk = nc.dram_tensor("msk", [PPAD, MF], F32, kind="ExternalInput")
    idn = nc.dram_tensor("idn", [PPAD, IDF], F16, kind="ExternalInput")
    out = nc.dram_tensor("out", [D, H, OF], F32, kind="ExternalOutput")
    with TileContext(nc) as tc:
        build_body(nc, tc, out.ap(), x3.ap(), msk.ap(), idn.ap())
    nc.finalize()
    return nc


def prep_x3(xb: np.ndarray, qb: int) -> np.ndarray:
    """xb [C,81,96,96] -> [128, 2*XF] fp16: x ~= h + 2^-11 * l2 (~2-ulp exact)."""
    q0 = qb * QB
    xs = np.zeros((PPAD, C, A * A, QQ), np.float32)
    lo, hi = max(PAD, q0), min(PAD + W, q0 + QQ)
    xs[PAD:PAD + H, :, :, lo - q0:hi - q0] = xb.transpose(2, 0, 1, 3)[:, :, :, lo - PAD:hi - PAD]
    h = xs.astype(np.float16)
    r = (xs - h.astype(np.float32)) * 2048.0
    l2 = r.astype(np.float16)
    out = np.concatenate([t.reshape(PPAD, XF) for t in (h, l2)], axis=1)
    return np.ascontiguousarray(out)


def prep_msk(mb: np.ndarray, qb: int) -> np.ndarray:
    """mb [81,96,96] -> [128, MF] (rows 96.. zero): m[p,(uv,q)] = mb[uv,p,q0+q]."""
    q0 = qb * QB
    m = np.zeros((PPAD, MF), np.float32)
    m[:H] = mb.transpose(1, 0, 2)[:, :, q0:q0 + QB].reshape(H, MF)
    return m


def prep_idn() -> np.ndarray:
    """One [128, IDB] diagonal strip per term; window at o gives I_off(o)."""
    idn = np.zeros((PPAD, NT, IDB), np.float16)
    k = np.arange(PAD, PAD + H)
    idn[k, 0, k] = 1.0
    idn[k, 1, k] = 2.0 ** -11
    return np.ascontiguousarray(idn.reshape(PPAD, IDF))


_IDN = None


def kernel(x: np.ndarray, mask: np.ndarray):
    global _IDN
    x = np.asarray(x, np.float32)
    mask = np.asarray(mask, np.float32)
    ctr = x[:, :, 40:41, :, :].copy()
    if _IDN is None:
        _IDN = prep_idn()
    nc = build_nc()
    in_maps = []
    for core in range(8):
        b, qb = divmod(core, 4)
        in_maps.append(
            {
                "x3": prep_x3(x[b], qb),
                "msk": prep_msk(mask[b], qb),
                "idn": _IDN,
            }
        )
    res = run_bass_kernel_spmd(nc, in_maps, list(range(8)))
    cost6 = np.empty((2, D, H, W, C, A * A), np.float32)
    for core in range(8):
        b, qb = divmod(core, 4)
        # device layout per d,p: (u, ch, c', v, q) -> reference (q, c, u, v)
        o = res.results[core]["out"].reshape(D, H, A, 2, 2, A, QB)
        o = o.transpose(0, 1, 6, 3, 4, 2, 5).reshape(D, H, QB, C, A * A)
        cost6[b, :, :, qb * QB:(qb + 1) * QB, :, :] = o
    cost = np.ascontiguousarray(cost6.reshape(2, D, H * W * C, A * A))
    return cost, ctr


# revision 38
# speedup vs baseline: 1.0458x; 1.0458x over previous
"""BuildCost kernel for Trainium2 (Bass/Tile), 8-core SPMD.

cost[b,d,p,q,c,uv] = xpad[b,c,u,v, p+off(d,u), q+off(d,v)] * mask[b,uv,p,q]
with off(d_idx,t) = d_idx*(4-t) + 4*t  (d_idx = d - mindisp), padded border 16.

Sharding: core = b*4 + qb  (b in {0,1}, qb in {0..3} -> q columns [qb*24, qb*24+24)).

Per core inputs (host pre-laid-out in SBUF geometry):
  x3  [128 pp, (t,c,uv,qq=56)] bf16: exact 3-term split x = h + l + ll
  msk [128 p,  (u,q,v)] f32 mask slice (rows 96.. zero)
  idn [128 k,  (off=33, m=96)] bf16 shifted identities (zero cols where the
      selected row is out of range -> borders need no zeroing on device)

Per (d,u,ch): three accumulating bf16 matmuls PSUM[p, q*18+c'*9+v] +=
IDN[off(d,u)].T @ x3[t, ...]; the row shift comes from the one-hot identity
(partition-legal), the column shift off(d,v) is folded into the rhs AP via
the linear-in-v stride 60-d_idx; rhs streams q contiguously (PE-fast) while
a permuted psum out-AP lands results in (q,c',v) order (DVE-fast).
Then OUT_d[p,(q,c,u,v)] = PSUM * mask on DVE (stride-0 broadcast over c',
all APs v-contiguous), and one contiguous 2.99MB DMA per d.

The bf16 triple split is exact: h,l,ll capture disjoint 8-bit significand
ranges of the fp32 value, and the fp32 PSUM accumulation of the three
one-hot matmul results reconstructs x bit-exactly.
"""

import numpy as np

from concourse import bacc, bass, mybir
from concourse.ap import AP
from concourse.tile import TileContext
from concourse.bass_utils import run_bass_kernel_spmd

A = 9            # angular resolution
D = 9            # disparities
H = W = 96
C = 4
PAD = 16
PPAD = 128       # padded row count
QB = W // 4      # q-block per core = 24
QQ = QB + 32     # stored qq window = 56
CUV = C * A * A  # 324
XF = C * A * A * QQ   # 18144  one split-term's free size
MF = A * A * QB       # 1944   mask free size, layout (uv, q)
NOFF = 33
IDW = 128             # identity columns = 128 to trigger FWL
NT = 2                # split terms: x = h + 2^-11 * l2 (fp16, ~2-ulp exact)
IDB = IDW + NOFF - 1  # 160: all 33 shifted identities are windows of one strip
IDF = NT * IDB        # 320 free elems: set 0 value 1.0, set 1 value 2^-11
OF = QB * CUV         # 7776   per-d output tile free size
NMM = 432             # matmul moving size = 2*9*24
F32 = mybir.dt.float32
F16 = mybir.dt.float16


def off(d_idx: int, t: int) -> int:
    return d_idx * (4 - t) + 4 * t


def _mkap(base: AP, extra_off: int, dims) -> AP:
    return AP(base.tensor, base.offset + extra_off, dims)


def build_body(nc: bass.Bass, tc, out: AP, x3: AP, msk: AP, idn: AP):
    with (
        tc.tile_pool(name="cst", bufs=1) as cst,
        tc.tile_pool(name="ob", bufs=3) as ob,
        tc.tile_pool(name="ps", bufs=1, space="PSUM") as ps,
    ):
        X3 = cst.tile([PPAD, NT * XF], F16, tag="X3")
        M0 = cst.tile([PPAD, MF], F32, tag="M0")
        IDN = cst.tile([PPAD, IDF], F16, tag="IDN")
        # split the big x load into (u, t) strips so the first matmul group
        # only waits for ~1MB instead of the whole 9.3MB; u=0 strips first,
        # then the small identity/mask tiles, then the rest
        strip = [[NT * XF, PPAD], [A * A * QQ, C], [1, A * QQ]]
        nc.sync.dma_start(IDN[:], idn)
        for t in range(NT):
            o = t * XF
            nc.sync.dma_start(_mkap(X3[:], o, strip), _mkap(x3, o, strip))
        for t in range(NT):
            o = t * XF + A * QQ
            nc.sync.dma_start(_mkap(X3[:], o, strip), _mkap(x3, o, strip))
        nc.sync.dma_start(M0[:], msk)
        for u in range(2, A):
            for t in range(NT):
                o = t * XF + u * A * QQ
                nc.sync.dma_start(_mkap(X3[:], o, strip), _mkap(x3, o, strip))

        for d_idx in range(D):
            Od = ob.tile([H, OF], F32, tag="Od")
            for u in range(A):
                o_u = off(d_idx, u)
                for ch in range(2):
                    # distinct cycling tags pin the 8 PSUM banks round-robin
                    gi = (d_idx * A + u) * 2 + ch
                    P = ps.tile([PPAD, NMM], F32, tag=f"ps{gi % 8}")
                    for t in range(NT):
                        # identity set t: values 1.0 (t=0) / 2^-11 (t=1);
                        # shifted identity = sliding window of the strip
                        lhsT = _mkap(
                            IDN[:], t * IDB + o_u, [[IDF, PPAD], [1, IDW]]
                        )
                        # rhs iteration (c'(2), v(9), q(24)): q contiguous
                        # innermost (PE-fast); v-stride 60-d_idx folds off(d,v)
                        rhs = _mkap(
                            X3[:],
                            t * XF + ch * 2 * (A * A * QQ) + u * A * QQ + 4 * d_idx,
                            [[NT * XF, PPAD], [A * A * QQ, 2], [60 - d_idx, A], [1, QB]],
                        )
                        nc.tensor.matmul(
                            P[:], lhsT, rhs, start=(t == 0), stop=(t == NT - 1)
                        )
                    # all APs iterate (c', v, q), q contiguous innermost; the
                    # device OUT layout is (u, ch, c', v, q) — the host permutes
                    # to the reference (q, c, u, v) order during assembly.
                    oap = _mkap(
                        Od[:], (u * 2 + ch) * NMM,
                        [[OF, H], [A * QB, 2], [QB, A], [1, QB]],
                    )
                    pap = _mkap(P[:], 0, [[NMM, H], [A * QB, 2], [QB, A], [1, QB]])
                    map_ = _mkap(
                        M0[:], u * A * QB,
                        [[MF, H], [0, 2], [QB, A], [1, QB]],
                    )
                    nc.any.tensor_mul(oap, pap, map_)
                if u % 3 == 2:
                    # drain this u-third of Od as soon as its 6 TTs are done
                    lo, hi = (u - 2) * 2 * NMM, (u + 1) * 2 * NMM
                    nc.sync.dma_start(
                        _mkap(out[d_idx], lo, [[OF, H], [1, hi - lo]]),
                        _mkap(Od[:], lo, [[OF, H], [1, hi - lo]]),
                    )


def build_nc() -> bass.Bass:
    nc = bacc.Bacc("TRN2", target_bir_lowering=False, debug=False)
    x3 = nc.dram_tensor("x3", [PPAD, NT * XF], F16, kind="ExternalInput")
    msk = nc.dram_tensor("msk", [PPAD, MF], F32, kind="ExternalInput")
    idn = nc.dram_tensor("idn", [PPAD, IDF], F16, kind="ExternalInput")
    out = nc.dram_tensor("out", [D, H, OF], F32, kind="ExternalOutput")
    with TileContext(nc) as tc:
        build_body(nc, tc, out.ap(), x3.ap(), msk.ap(), idn.ap())
    nc.finalize()
    return nc


def prep_x3(xb: np.ndarray, qb: int) -> np.ndarray:
    """xb [C,81,96,96] -> [128, 2*XF] fp16: x ~= h + 2^-11 * l2 (~2-ulp exact)."""
    q0 = qb * QB
    xs = np.zeros((PPAD, C, A * A, QQ), np.float32)
    lo, hi = max(PAD, q0), min(PAD + W, q0 + QQ)
    xs[PAD:PAD + H, :, :, lo - q0:hi - q0] = xb.transpose(2, 0, 1, 3)[:, :, :, lo - PAD:hi - PAD]
    h = xs.astype(np.float16)
    r = (xs - h.astype(np.float32)) * 2048.0
    l2 = r.astype(np.float16)
    out = np.concatenate([t.reshape(PPAD, XF) for t in (h, l2)], axis=1)
    return np.ascontiguousarray(out)


def prep_msk(mb: np.ndarray, qb: int) -> np.ndarray:
    """mb [81,96,96] -> [128, MF] (rows 96.. zero): m[p,(uv,q)] = mb[uv,p,q0+q]."""
    q0 = qb * QB
    m = np.zeros((PPAD, MF), np.float32)
    m[:H] = mb.transpose(1, 0, 2)[:, :, q0:q0 + QB].reshape(H, MF)
    return m


def prep_idn() -> np.ndarray:
    """One [128, IDB] diagonal strip per term; window at o gives I_off(o)."""
    idn = np.zeros((PPAD, NT, IDB), np.float16)
    k = np.arange(PAD, PAD + H)
    idn[k, 0, k] = 1.0
    idn[k, 1, k] = 2.0 ** -11
    return np.ascontiguousarray(idn.reshape(PPAD, IDF))


_IDN = None


def kernel(x: np.ndarray, mask: np.ndarray):
    global _IDN
    x = np.asarray(x, np.float32)
    mask = np.asarray(mask, np.float32)
    ctr = x[:, :, 40:41, :, :].copy()
    if _IDN is None:
        _IDN = prep_idn()
    nc = build_nc()
    in_maps = []
    for core in range(8):
        b, qb = divmod(core, 4)
        in_maps.append(
            {
                "x3": prep_x3(x[b], qb),
                "msk": prep_msk(mask[b], qb),
                "idn": _IDN,
            }
        )
    res = run_bass_kernel_spmd(nc, in_maps, list(range(8)))
    cost6 = np.empty((2, D, H, W, C, A * A), np.float32)
    for core in range(8):
        b, qb = divmod(core, 4)
        # device layout per d,p: (u, ch, c', v, q) -> reference (q, c, u, v)
        o = res.results[core]["out"].reshape(D, H, A, 2, 2, A, QB)
        o = o.transpose(0, 1, 6, 3, 4, 2, 5).reshape(D, H, QB, C, A * A)
        cost6[b, :, :, qb * QB:(qb + 1) * QB, :, :] = o
    cost = np.ascontiguousarray(cost6.reshape(2, D, H * W * C, A * A))
    return cost, ctr
